# revision 10
# baseline (speedup 1.0000x reference)
"""CAM_Module (channel attention) Trainium2 Bass kernel, v3 (fp8 MM2).

x: (16, 512, 64, 64) f32, gamma: (1,) f32
  xf = x.reshape(B, C, N)           N = 4096
  energy = xf @ xf^T                (B, C, C)
  att = softmax(max(energy) - energy, axis=-1)   == softmax(-energy) (shift-invariant)
  out = gamma * (att @ xf) + x

Sharding: data-parallel over batch, 2 batches per core on 8 cores.

v3 design (vs v2):
  - MM2 runs fp8e4 DoubleRow (2 matmuls/chunk over paired c-tiles), cutting
    MM2 PE time ~1.4x; PE total drops below the ~94us HBM roofline
  - xf is cast bf16 -> fp8 pair tiles on GpSimd (idle on HW) per load chunk
  - attT drops the folded +I: residual now rides the PSUM->SBUF eviction as
    a DVE tensor_add(psum, xf_bf16) -> f32, same cost as the old copy
  - aT stored as fp8 [128, 2, C] pair tiles = DoubleRow lhsT layout
  - store stream starts mid b1-MM1 (~when loads drain) so the DMA queues
    stay saturated end to end
"""

import sys

if "/opt/trn_rl_repo" not in sys.path:
    sys.path.insert(0, "/opt/trn_rl_repo")

from contextlib import ExitStack

import numpy as np

import concourse.bass as bass
import concourse.tile as tile
from concourse import bacc, mybir
from concourse.masks import make_identity

N_CORES = 8
B, C, H, W = 16, 512, 64, 64
N = H * W                    # 4096
BPC = B // N_CORES           # batches per core = 2
CT = C // 128                # 4 c-tiles
NP = CT // 2                 # c-tile pairs (DoubleRow contraction)
KT = N // 128                # 32 k-chunks (transposed layout)

F32 = mybir.dt.float32
F32R = mybir.dt.float32r
BF16 = mybir.dt.bfloat16
F8 = mybir.dt.float8e4
DR = mybir.MatmulPerfMode.DoubleRow

LAG = 4                      # tr -> MM1 pipeline depth (k-chunks)
PFX = 6                      # b1 transposes emitted before softmax(0)


def _build_nc(reps=1):
    nc = bacc.Bacc("TRN2", target_bir_lowering=False, debug=False,
                   num_devices=N_CORES)
    x_d = nc.dram_tensor("x", [BPC, C, N], F32, kind="ExternalInput").ap()
    g_d = nc.dram_tensor("gamma", [1], F32, kind="ExternalInput").ap()
    o_d = nc.dram_tensor("out", [BPC, C, N], F32, kind="ExternalOutput").ap()

    with tile.TileContext(nc) as tc, ExitStack() as ctx:
        xf_pool = ctx.enter_context(tc.tile_pool(name="xf", bufs=BPC * CT))
        xf8_pool = ctx.enter_context(tc.tile_pool(name="xf8", bufs=BPC * NP))
        xfT_pool = ctx.enter_context(tc.tile_pool(name="xfT", bufs=LAG + 4))
        att_pool = ctx.enter_context(tc.tile_pool(name="att", bufs=2 * CT))
        attT_pool = ctx.enter_context(tc.tile_pool(name="attT", bufs=2 * NP))
        d_pool = ctx.enter_context(tc.tile_pool(name="dsc", bufs=2 * CT))
        mir_pool = ctx.enter_context(tc.tile_pool(name="mir", bufs=3))
        out_pool = ctx.enter_context(tc.tile_pool(name="outp", bufs=10))
        stat_pool = ctx.enter_context(tc.tile_pool(name="stat", bufs=4 * CT))
        one_pool = ctx.enter_context(tc.tile_pool(name="one", bufs=1))
        pT = ctx.enter_context(tc.tile_pool(name="pT", bufs=2, space="PSUM"))
        pE = ctx.enter_context(tc.tile_pool(name="pE", bufs=CT, space="PSUM"))
        pO = ctx.enter_context(tc.tile_pool(name="pO", bufs=2, space="PSUM"))

        # identities for PE transpose-mode: f32 master, bf16 + f32r copies
        ident_f = one_pool.tile([128, 128], F32, tag="idf")
        make_identity(nc, ident_f[:])
        ident = one_pool.tile([128, 128], BF16, tag="idb")
        nc.vector.tensor_copy(ident[:], ident_f[:])
        ident_r = one_pool.tile([128, 128], F32R, tag="idr")
        nc.vector.tensor_copy(ident_r[:], ident_f[:])

        # HAM warmup: ~2.5us of dummy matmuls fill the initial DMA wait
        # and bring the PE clock gate to 8/8 before the real transposes.
        # Outside the rep loop: across reps PE idle gaps stay < 3.4us so
        # the clock gate never re-throttles.
        wu = pT.tile([128, 128], F32, tag="pt", name="wu")
        for i in range(24):
            nc.tensor.matmul(wu[:], ident[:], ident[:], start=True,
                             stop=True)

        # broadcast gamma to all 128 partitions via K=1 matmul with ones
        # (after the warmup so the ~2.4us g_sb DMA doesn't head-block PE)
        g_sb = one_pool.tile([1, 1], F32, tag="gsb")
        nc.sync.dma_start(g_sb[:], g_d.rearrange("(a b) -> a b", a=1))
        ones = one_pool.tile([1, 128], F32, tag="ones")
        nc.vector.memset(ones[:], 1.0)
        pG = pT.tile([128, 1], F32, tag="pt", name="pG")
        nc.tensor.matmul(pG[:], ones[:], g_sb[:], start=True, stop=True)
        g_bc = one_pool.tile([128, 1], F32, tag="gbc")
        nc.vector.tensor_copy(g_bc[:], pG[:])

        loop_ctx = tc.For_i(0, reps, 1) if reps > 1 else None
        if loop_ctx is not None:
            ctx.enter_context(loop_ctx)

        # per-c-tile load chunks; chunk 0 is issued as two sub-DMAs
        # (128 + 384 cols) so the first transposes can start early.
        # SWDGE has ~1us fixed cost per dma_start, so later chunks are big.
        CHUNKS = [(0, 512), (512, 512), (1024, 1024), (2048, 1024),
                  (3072, 1024)]

        def chunk_of(col):
            for i, (off, w) in enumerate(CHUNKS):
                if off <= col < off + w:
                    return i, col - off
            raise AssertionError(col)

        st = [dict() for _ in range(BPC)]

        def emit_loads(b):
            s = st[b]
            s["xf"] = [[None] * len(CHUNKS) for _ in range(CT)]
            for q in range(len(CHUNKS)):
                off, w = CHUNKS[q]
                for ct in range(CT):
                    t = xf_pool.tile([128, w], BF16, tag=f"xf{q}",
                                     name=f"xf_{b}_{ct}_{q}")
                    if q == 0:
                        nc.gpsimd.dma_start(
                            t[:, 0:128],
                            x_d[b, ct * 128:(ct + 1) * 128, 0:128])
                        nc.gpsimd.dma_start(
                            t[:, 128:512],
                            x_d[b, ct * 128:(ct + 1) * 128, 128:512])
                    else:
                        nc.gpsimd.dma_start(
                            t[:],
                            x_d[b, ct * 128:(ct + 1) * 128, off:off + w])
                    s["xf"][ct][q] = t

        def emit_cast8(b):
            # xf8[a] = fp8 pair tile [128, 2, N]: o=0 -> c-tile 2a,
            # o=1 -> c-tile 2a+1.  GpSimd copies per load chunk so the
            # cast stream rides ~2us behind the load stream.
            s = st[b]
            s["xf8"] = [
                xf8_pool.tile([128, 2, N], F8, tag="xf8",
                              name=f"xf8_{b}_{a}")
                for a in range(NP)
            ]
            for q in range(len(CHUNKS)):
                off, w = CHUNKS[q]
                for ct in range(CT):
                    a, o = divmod(ct, 2)
                    nc.gpsimd.tensor_copy(
                        s["xf8"][a][:, o, off:off + w],
                        s["xf"][ct][q][:],
                    )

        def xf_slice(b, ct, col, width):
            q, o = chunk_of(col)
            return st[b]["xf"][ct][q][:, o:o + width]

        def emit_tr(b, k):
            tp = pT.tile([128, C], BF16, tag="pt", name=f"tp_{b}_{k}")
            for ct in range(CT):
                nc.tensor.transpose(
                    tp[:, ct * 128:(ct + 1) * 128],
                    xf_slice(b, ct, k * 128, 128),
                    ident[:],
                )
            xT = xfT_pool.tile([128, C], BF16, tag="xT", name=f"xT_{b}_{k}")
            if k % 2 == 0:
                nc.vector.tensor_copy(xT[:], tp[:])
            else:
                nc.scalar.copy(xT[:], tp[:])
            return xT

        def emit_mm1(b, k, xT):
            # energy is symmetric: compute only j >= i blocks (shrinking
            # moving width per i-tile); lower blocks are mirrored after
            for it in range(CT):
                nc.tensor.matmul(
                    st[b]["e"][it][:, it * 128:C],
                    xT[:, it * 128:(it + 1) * 128],
                    xT[:, it * 128:C],
                    start=(k == 0),
                    stop=(k == KT - 1),
                )

        def emit_trmm1(b, k_from=0, prefix=(), interleave=None):
            s = st[b]
            s["e"] = [
                pE.tile([128, C], F32, tag="pe", name=f"pe_{b}_{i}")
                for i in range(CT)
            ]
            pending = list(prefix)
            for k in range(k_from, KT):
                pending.append(emit_tr(b, k))
                if len(pending) > LAG:
                    emit_mm1(b, k - len(pending) + 1, pending.pop(0))
                if interleave is not None:
                    interleave(k)
            base = KT - len(pending)
            for i, xT in enumerate(pending):
                emit_mm1(b, base + i, xT)

        def emit_mirror(b):
            # mirror lower-triangle blocks e[t][:, u] = e[u][:, t].T via
            # sbuf bounce + transpose into a scratch psum bank + ACT
            # write-back (PE never touches accumulation-grouped banks);
            # f32r keeps the mirrored energies bit-exact
            e_ps = st[b]["e"]
            for t in range(1, CT):
                mp = pT.tile([128, C], F32R, tag="pt", name=f"mp_{b}_{t}")
                for u in range(t):
                    mtmp = mir_pool.tile([128, 128], F32R, tag="mir",
                                         name=f"mir_{b}_{t}_{u}")
                    nc.scalar.copy(
                        mtmp[:], e_ps[u][:, t * 128:(t + 1) * 128])
                    nc.tensor.transpose(
                        mp[:, u * 128:(u + 1) * 128], mtmp[:], ident_r[:])
                nc.scalar.copy(
                    e_ps[t][:, 0:t * 128], mp[:, 0:t * 128])

        def emit_softmax(b, it):
            # att row i = exp(min_i - e_i); the 1/Z*gamma scale is deferred
            # to the attT transpose via D = diag(rz*g)
            s = st[b]
            if it == 0:
                s["att"] = [None] * CT
                s["D"] = [None] * CT
            m = stat_pool.tile([128, 1], F32, tag="m", name=f"m_{b}_{it}")
            nc.vector.tensor_reduce(
                m[:], s["e"][it][:], axis=mybir.AxisListType.X,
                op=mybir.AluOpType.min,
            )
            a = att_pool.tile([128, C], BF16, tag="a", name=f"a_{b}_{it}")
            z = stat_pool.tile([128, 1], F32, tag="z", name=f"z_{b}_{it}")
            nc.scalar.activation(
                a[:], s["e"][it][:], mybir.ActivationFunctionType.Exp,
                bias=m[:], scale=-1.0, accum_out=z[:],
            )
            rz = stat_pool.tile([128, 1], F32, tag="rz", name=f"rz_{b}_{it}")
            nc.vector.reciprocal(rz[:], z[:])
            g = stat_pool.tile([128, 1], F32, tag="g", name=f"g_{b}_{it}")
            nc.vector.tensor_mul(g[:], rz[:], g_bc[:])
            D = d_pool.tile([128, 128], BF16, tag="D", name=f"D_{b}_{it}")
            nc.vector.tensor_scalar_mul(D[:], ident[:], g[:])
            s["att"][it] = a
            s["D"][it] = D

        def emit_attT8(b, pair):
            # aT8[a][:, o, :] = (att.T * colscale) for c-tile jt = 2a+o,
            # cast to fp8 = DoubleRow lhsT pair layout.  No +I fold: the
            # residual is added at PSUM eviction in emit_mm2_chunk.
            s = st[b]
            if "attT8" not in s:
                s["attT8"] = [
                    attT_pool.tile([128, 2, C], F8, tag="aT8",
                                   name=f"aT8_{b}_{aa}")
                    for aa in range(NP)
                ]
            for o in range(2):
                jt = 2 * pair + o
                tp = pT.tile([128, C], F32, tag="pt", name=f"at_{b}_{jt}")
                for it in range(CT):
                    nc.tensor.matmul(
                        tp[:, it * 128:(it + 1) * 128],
                        s["att"][it][:, jt * 128:(jt + 1) * 128],
                        s["D"][it][:],
                        start=True,
                        stop=True,
                    )
                if jt % 2 == 0:
                    nc.vector.tensor_copy(s["attT8"][pair][:, o, :], tp[:])
                else:
                    nc.scalar.copy(s["attT8"][pair][:, o, :], tp[:])

        MM2_BANKS4 = ["po", "po", "pt", "pt"]
        MM2_BANKS8 = ["po", "po", "pt", "pt", "pe", "pe", "pe", "pe"]

        def emit_mm2_chunk(b, it, nch, ci, banks=None):
            # out[it, nch] = gamma*(att @ xf)[it, nch] + x[it, nch]:
            #   2 fp8 DoubleRow matmuls (c-tile pairs) accumulate the
            #   attention part in PSUM; the residual rides the eviction
            #   as a DVE tensor_add with the bf16 xf tile.
            # rotate over PSUM banks (pO's 2 + pT's 2 + retired pE's 4
            # when free) so matmuls never wait on eviction latency
            s = st[b]
            tag = banks[ci % len(banks)] if banks else "po"
            pool = {"po": pO, "pt": pT, "pe": pE}[tag]
            po = pool.tile([128, 512], F32, tag=tag,
                           name=f"po_{b}_{it}_{nch}")
            for a in range(NP):
                nc.tensor.matmul(
                    po[:],
                    s["attT8"][a][:, :, it * 128:(it + 1) * 128],
                    s["xf8"][a][:, :, nch * 512:(nch + 1) * 512],
                    start=(a == 0),
                    stop=(a == NP - 1),
                    perf_mode=DR,
                )
            o_t = out_pool.tile([128, 512], F32, tag="o",
                                name=f"o_{b}_{it}_{nch}")
            nc.vector.tensor_add(
                o_t[:], po[:], xf_slice(b, it, nch * 512, 512))
            nc.sync.dma_start(
                o_d[b, it * 128:(it + 1) * 128,
                    nch * 512:(nch + 1) * 512],
                o_t[:],
            )

        def emit_mm2(b, chunks=None, interleave=None, banks=MM2_BANKS4):
            for ci, (it, nch) in enumerate(
                    chunks if chunks is not None else
                    [(i, n) for i in range(CT) for n in range(N // 512)]):
                emit_mm2_chunk(b, it, nch, ci, banks=banks)
                if interleave is not None:
                    interleave(ci)

        # ---- emission schedule ----
        emit_loads(0)
        emit_loads(1)
        emit_cast8(0)
        emit_cast8(1)
        emit_trmm1(0)
        emit_mirror(0)
        emit_softmax(0, 0)  # e[0] needs no mirror; unblocks b1's MM1 early
        pfx = [emit_tr(1, k) for k in range(PFX)]

        ALL_CHUNKS = [(i, n) for i in range(CT) for n in range(N // 512)]

        def ilv_sm0(k):
            # softmax(0), attT8(0), then the first mm2(0) chunks ride
            # inside b1's tr+MM1 phase: PE has buffered mm2 work to chew
            # while the MM1 tail waits on b1's final (DMA-starved) loads,
            # and the output store stream starts before loads fully drain
            if PFX + 1 <= k <= PFX + 3:
                emit_softmax(0, k - PFX)
            elif k in (10, 12):
                emit_attT8(0, (k - 10) // 2)
            elif k >= 14:
                it, nch = ALL_CHUNKS[k - 14]
                emit_mm2_chunk(0, it, nch, ci=k, banks=["po"])

        emit_trmm1(1, k_from=PFX, prefix=pfx, interleave=ilv_sm0)
        emit_mirror(1)

        def ilv_sm1(ci):
            # softmax(1) rides inside b0's MM2 phase
            if 1 <= ci <= 4:
                emit_softmax(1, ci - 1)

        def ilv_sm1b(ci):
            # attT8(1) once all of b1's energy has been consumed
            if ci in (0, 2):
                emit_attT8(1, ci // 2)

        emit_mm2(0, chunks=ALL_CHUNKS[18:26], interleave=ilv_sm1,
                 banks=MM2_BANKS4)
        emit_mm2(0, chunks=ALL_CHUNKS[26:], interleave=ilv_sm1b,
                 banks=MM2_BANKS8)
        emit_mm2(1, banks=MM2_BANKS8)

    nc.compile()
    return nc


_RUNNER = None


def _build_runner(nc=None):
    """Compile once; return a callable (xf_full, gamma) -> out_full.

    Mirrors concourse.bass2jax.run_bass_via_pjrt but caches the jitted
    shard_map executable so repeated kernel() calls don't re-lower, and
    keeps the output-seed zero buffers resident on device.
    """
    import jax
    from jax.sharding import Mesh, NamedSharding, PartitionSpec
    from jax.experimental.shard_map import shard_map

    from concourse import bass2jax, mybir as _mybir
    from concourse.bass2jax import _bass_exec_p, partition_id_tensor

    if nc is None:
        nc = _build_nc()
    bass2jax.install_neuronx_cc_hook()

    partition_name = (
        nc.partition_id_tensor.name if nc.partition_id_tensor else None
    )
    in_names, out_names, out_avals, zero_shapes = [], [], [], []
    for alloc in nc.m.functions[0].allocations:
        if not isinstance(alloc, _mybir.MemoryLocationSet):
            continue
        name = alloc.memorylocations[0].name
        if alloc.kind == "ExternalInput":
            if name != partition_name:
                in_names.append(name)
        elif alloc.kind == "ExternalOutput":
            shape = tuple(alloc.tensor_shape)
            dtype = _mybir.dt.np(alloc.dtype)
            out_names.append(name)
            out_avals.append(jax.core.ShapedArray(shape, dtype))
            zero_shapes.append((shape, dtype))
    n_params = len(in_names)
    all_names = list(in_names) + list(out_names)
    if partition_name is not None:
        all_names.append(partition_name)

    def _body(*args):
        operands = list(args)
        if partition_name is not None:
            operands.append(partition_id_tensor())
        return tuple(
            _bass_exec_p.bind(
                *operands,
                out_avals=tuple(out_avals),
                in_names=tuple(all_names),
                out_names=tuple(out_names),
                lowering_input_output_aliases=(),
                sim_require_finite=True,
                sim_require_nnan=True,
                nc=nc,
            )
        )

    devices = jax.devices()[:N_CORES]
    mesh = Mesh(np.asarray(devices), ("core",))
    n_in = n_params + len(out_names)
    sharded = jax.jit(
        shard_map(
            _body,
            mesh=mesh,
            in_specs=(PartitionSpec("core"),) * n_in,
            out_specs=(PartitionSpec("core"),) * len(out_names),
            check_rep=False,
        ),
        keep_unused=True,
    )

    # in_names order is discovered from allocations; map our two inputs
    assert set(in_names) == {"x", "gamma"}, in_names

    # output-seed buffers created on device once (kernel writes out fully)
    sh = NamedSharding(mesh, PartitionSpec("core"))
    zeros_dev = [
        jax.jit(
            lambda s=s, d=d: jax.numpy.zeros((N_CORES * s[0],) + s[1:], d),
            out_shardings=sh,
        )()
        for s, d in zero_shapes
    ]
    jax.block_until_ready(zeros_dev)

    def run(xf_full, gamma):
        per_in = {
            "x": xf_full,  # (16, 512, 4096) == concat of per-core (2, 512, 4096)
            "gamma": np.ascontiguousarray(
                np.broadcast_to(np.asarray(gamma, np.float32).reshape(1),
                                (N_CORES,))
            ),
        }
        concat_in = [per_in[name] for name in in_names]
        out_arrs = sharded(*concat_in, *zeros_dev)
        return np.asarray(out_arrs[out_names.index("out")])

    run.sharded = sharded
    run.zeros_dev = zeros_dev
    run.in_names = in_names
    run.out_names = out_names
    run.mesh = mesh
    return run


def _get_runner():
    global _RUNNER
    if _RUNNER is None:
        _RUNNER = _build_runner()
    return _RUNNER


def kernel(x, gamma):
    assert x.shape == (B, C, H, W)
    run = _get_runner()
    xf = np.ascontiguousarray(np.asarray(x, np.float32).reshape(B, C, N))
    g = np.asarray(gamma, np.float32)
    out = run(xf, g)
    return out.reshape(B, C, H, W).astype(np.float32, copy=False)


# revision 13
# speedup vs baseline: 1.5518x; 1.5518x over previous
"""CAM_Module (channel attention) Trainium2 Bass kernel, v3 (fp8 MM2).

x: (16, 512, 64, 64) f32, gamma: (1,) f32
  xf = x.reshape(B, C, N)           N = 4096
  energy = xf @ xf^T                (B, C, C)
  att = softmax(max(energy) - energy, axis=-1)   == softmax(-energy) (shift-invariant)
  out = gamma * (att @ xf) + x

Sharding: data-parallel over batch, 2 batches per core on 8 cores.

v3 design (vs v2):
  - MM2 runs fp8e4 DoubleRow (2 matmuls/chunk over paired c-tiles), cutting
    MM2 PE time ~1.4x; PE total drops below the ~94us HBM roofline
  - xf is cast bf16 -> fp8 pair tiles on GpSimd (idle on HW) per load chunk
  - attT drops the folded +I: residual now rides the PSUM->SBUF eviction as
    a DVE tensor_add(psum, xf_bf16) -> f32, same cost as the old copy
  - aT stored as fp8 [128, 2, C] pair tiles = DoubleRow lhsT layout
  - store stream starts mid b1-MM1 (~when loads drain) so the DMA queues
    stay saturated end to end
"""

import sys

if "/opt/trn_rl_repo" not in sys.path:
    sys.path.insert(0, "/opt/trn_rl_repo")

from contextlib import ExitStack

import numpy as np

import concourse.bass as bass
import concourse.tile as tile
from concourse import bacc, mybir
from concourse.masks import make_identity

N_CORES = 8
B, C, H, W = 16, 512, 64, 64
N = H * W                    # 4096
BPC = B // N_CORES           # batches per core = 2
CT = C // 128                # 4 c-tiles
NP = CT // 2                 # c-tile pairs (DoubleRow contraction)
KT = N // 128                # 32 k-chunks (transposed layout)

F32 = mybir.dt.float32
F32R = mybir.dt.float32r
BF16 = mybir.dt.bfloat16
F8 = mybir.dt.float8e4
DR = mybir.MatmulPerfMode.DoubleRow

LAG = 4                      # tr -> MM1 pipeline depth (k-chunks)
PFX = 6                      # b1 transposes emitted before softmax(0)


def _build_nc(reps=1):
    nc = bacc.Bacc("TRN2", target_bir_lowering=False, debug=False,
                   num_devices=N_CORES)
    x_d = nc.dram_tensor("x", [BPC, C, N], F32, kind="ExternalInput").ap()
    g_d = nc.dram_tensor("gamma", [1], F32, kind="ExternalInput").ap()
    o_d = nc.dram_tensor("out", [BPC, C, N], F32, kind="ExternalOutput").ap()

    with tile.TileContext(nc) as tc, ExitStack() as ctx:
        xf_pool = ctx.enter_context(tc.tile_pool(name="xf", bufs=BPC * CT))
        xf8_pool = ctx.enter_context(tc.tile_pool(name="xf8", bufs=BPC * NP))
        xfT_pool = ctx.enter_context(tc.tile_pool(name="xfT", bufs=LAG + 4))
        att_pool = ctx.enter_context(tc.tile_pool(name="att", bufs=2 * CT))
        attT_pool = ctx.enter_context(tc.tile_pool(name="attT", bufs=2 * NP))
        d_pool = ctx.enter_context(tc.tile_pool(name="dsc", bufs=2 * CT))
        mir_pool = ctx.enter_context(tc.tile_pool(name="mir", bufs=3))
        out_pool = ctx.enter_context(tc.tile_pool(name="outp", bufs=10))
        stat_pool = ctx.enter_context(tc.tile_pool(name="stat", bufs=4 * CT))
        one_pool = ctx.enter_context(tc.tile_pool(name="one", bufs=1))
        pT = ctx.enter_context(tc.tile_pool(name="pT", bufs=2, space="PSUM"))
        pE = ctx.enter_context(tc.tile_pool(name="pE", bufs=CT, space="PSUM"))
        pO = ctx.enter_context(tc.tile_pool(name="pO", bufs=2, space="PSUM"))

        # identities for PE transpose-mode: f32 master, bf16 + f32r copies
        ident_f = one_pool.tile([128, 128], F32, tag="idf")
        make_identity(nc, ident_f[:])
        ident = one_pool.tile([128, 128], BF16, tag="idb")
        nc.vector.tensor_copy(ident[:], ident_f[:])
        ident_r = one_pool.tile([128, 128], F32R, tag="idr")
        nc.vector.tensor_copy(ident_r[:], ident_f[:])

        # HAM warmup: ~2.5us of dummy matmuls fill the initial DMA wait
        # and bring the PE clock gate to 8/8 before the real transposes.
        # Outside the rep loop: across reps PE idle gaps stay < 3.4us so
        # the clock gate never re-throttles.
        wu = pT.tile([128, 128], F32, tag="pt", name="wu")
        for i in range(24):
            nc.tensor.matmul(wu[:], ident[:], ident[:], start=True,
                             stop=True)

        # broadcast gamma to all 128 partitions via K=1 matmul with ones
        # (after the warmup so the ~2.4us g_sb DMA doesn't head-block PE)
        g_sb = one_pool.tile([1, 1], F32, tag="gsb")
        nc.sync.dma_start(g_sb[:], g_d.rearrange("(a b) -> a b", a=1))
        ones = one_pool.tile([1, 128], F32, tag="ones")
        nc.vector.memset(ones[:], 1.0)
        pG = pT.tile([128, 1], F32, tag="pt", name="pG")
        nc.tensor.matmul(pG[:], ones[:], g_sb[:], start=True, stop=True)
        g_bc = one_pool.tile([128, 1], F32, tag="gbc")
        nc.vector.tensor_copy(g_bc[:], pG[:])

        loop_ctx = tc.For_i(0, reps, 1) if reps > 1 else None
        if loop_ctx is not None:
            ctx.enter_context(loop_ctx)

        # per-c-tile load chunks; chunk 0 is issued as two sub-DMAs
        # (128 + 384 cols) so the first transposes can start early.
        # SWDGE has ~1us fixed cost per dma_start, so later chunks are big.
        CHUNKS = [(0, 512), (512, 512), (1024, 1024), (2048, 1024),
                  (3072, 1024)]

        def chunk_of(col):
            for i, (off, w) in enumerate(CHUNKS):
                if off <= col < off + w:
                    return i, col - off
            raise AssertionError(col)

        st = [dict() for _ in range(BPC)]

        def emit_loads(b):
            s = st[b]
            s["xf"] = [[None] * len(CHUNKS) for _ in range(CT)]
            for q in range(len(CHUNKS)):
                off, w = CHUNKS[q]
                for ct in range(CT):
                    t = xf_pool.tile([128, w], BF16, tag=f"xf{q}",
                                     name=f"xf_{b}_{ct}_{q}")
                    if q == 0:
                        nc.gpsimd.dma_start(
                            t[:, 0:128],
                            x_d[b, ct * 128:(ct + 1) * 128, 0:128])
                        nc.gpsimd.dma_start(
                            t[:, 128:512],
                            x_d[b, ct * 128:(ct + 1) * 128, 128:512])
                    else:
                        nc.gpsimd.dma_start(
                            t[:],
                            x_d[b, ct * 128:(ct + 1) * 128, off:off + w])
                    s["xf"][ct][q] = t

        def alloc_xf8(b):
            # xf8[a] = fp8 pair tile [128, 2, N]: o=0 -> c-tile 2a,
            # o=1 -> c-tile 2a+1 (DoubleRow rhs pair layout)
            st[b]["xf8"] = [
                xf8_pool.tile([128, 2, N], F8, tag="xf8",
                              name=f"xf8_{b}_{a}")
                for a in range(NP)
            ]

        def emit_cast8(b, q):
            # cast one load chunk to fp8 across all 4 c-tiles, split
            # DVE/ACT; emitted right after that chunk's loads land so the
            # casts never head-block either queue
            s = st[b]
            off, w = CHUNKS[q]
            for ct in range(CT):
                a, o = divmod(ct, 2)
                dst = s["xf8"][a][:, o, off:off + w]
                if ct % 2 == 0:
                    nc.vector.tensor_copy(dst, s["xf"][ct][q][:])
                else:
                    nc.scalar.copy(dst, s["xf"][ct][q][:])

        def xf_slice(b, ct, col, width):
            q, o = chunk_of(col)
            return st[b]["xf"][ct][q][:, o:o + width]

        def emit_tr(b, k):
            tp = pT.tile([128, C], BF16, tag="pt", name=f"tp_{b}_{k}")
            for ct in range(CT):
                nc.tensor.transpose(
                    tp[:, ct * 128:(ct + 1) * 128],
                    xf_slice(b, ct, k * 128, 128),
                    ident[:],
                )
            xT = xfT_pool.tile([128, C], BF16, tag="xT", name=f"xT_{b}_{k}")
            if k % 2 == 0:
                nc.vector.tensor_copy(xT[:], tp[:])
            else:
                nc.scalar.copy(xT[:], tp[:])
            return xT

        def emit_mm1(b, k, xT):
            # energy is symmetric: compute only j >= i blocks (shrinking
            # moving width per i-tile); lower blocks are mirrored after
            for it in range(CT):
                nc.tensor.matmul(
                    st[b]["e"][it][:, it * 128:C],
                    xT[:, it * 128:(it + 1) * 128],
                    xT[:, it * 128:C],
                    start=(k == 0),
                    stop=(k == KT - 1),
                )

        def emit_trmm1(b, k_from=0, prefix=(), interleave=None):
            s = st[b]
            s["e"] = [
                pE.tile([128, C], F32, tag="pe", name=f"pe_{b}_{i}")
                for i in range(CT)
            ]
            pending = list(prefix)
            for k in range(k_from, KT):
                pending.append(emit_tr(b, k))
                if len(pending) > LAG:
                    emit_mm1(b, k - len(pending) + 1, pending.pop(0))
                if interleave is not None:
                    interleave(k)
            base = KT - len(pending)
            for i, xT in enumerate(pending):
                emit_mm1(b, base + i, xT)

        def emit_mirror(b):
            # mirror lower-triangle blocks e[t][:, u] = e[u][:, t].T via
            # sbuf bounce + transpose into a scratch psum bank + ACT
            # write-back (PE never touches accumulation-grouped banks);
            # f32r keeps the mirrored energies bit-exact
            e_ps = st[b]["e"]
            for t in range(1, CT):
                mp = pT.tile([128, C], F32R, tag="pt", name=f"mp_{b}_{t}")
                for u in range(t):
                    mtmp = mir_pool.tile([128, 128], F32R, tag="mir",
                                         name=f"mir_{b}_{t}_{u}")
                    nc.scalar.copy(
                        mtmp[:], e_ps[u][:, t * 128:(t + 1) * 128])
                    nc.tensor.transpose(
                        mp[:, u * 128:(u + 1) * 128], mtmp[:], ident_r[:])
                nc.scalar.copy(
                    e_ps[t][:, 0:t * 128], mp[:, 0:t * 128])

        def emit_softmax(b, it):
            # att row i = exp(min_i - e_i); the 1/Z*gamma scale is deferred
            # to the attT transpose via D = diag(rz*g)
            s = st[b]
            if it == 0:
                s["att"] = [None] * CT
                s["D"] = [None] * CT
            m = stat_pool.tile([128, 1], F32, tag="m", name=f"m_{b}_{it}")
            nc.vector.tensor_reduce(
                m[:], s["e"][it][:], axis=mybir.AxisListType.X,
                op=mybir.AluOpType.min,
            )
            a = att_pool.tile([128, C], BF16, tag="a", name=f"a_{b}_{it}")
            z = stat_pool.tile([128, 1], F32, tag="z", name=f"z_{b}_{it}")
            nc.scalar.activation(
                a[:], s["e"][it][:], mybir.ActivationFunctionType.Exp,
                bias=m[:], scale=-1.0, accum_out=z[:],
            )
            rz = stat_pool.tile([128, 1], F32, tag="rz", name=f"rz_{b}_{it}")
            nc.vector.reciprocal(rz[:], z[:])
            g = stat_pool.tile([128, 1], F32, tag="g", name=f"g_{b}_{it}")
            nc.vector.tensor_mul(g[:], rz[:], g_bc[:])
            D = d_pool.tile([128, 128], BF16, tag="D", name=f"D_{b}_{it}")
            nc.vector.tensor_scalar_mul(D[:], ident[:], g[:])
            s["att"][it] = a
            s["D"][it] = D

        def emit_attT8(b, pair):
            # aT8[a][:, o, :] = (att.T * colscale) for c-tile jt = 2a+o,
            # cast to fp8 = DoubleRow lhsT pair layout.  No +I fold: the
            # residual is added at PSUM eviction in emit_mm2_chunk.
            s = st[b]
            if "attT8" not in s:
                s["attT8"] = [
                    attT_pool.tile([128, 2, C], F8, tag="aT8",
                                   name=f"aT8_{b}_{aa}")
                    for aa in range(NP)
                ]
            for o in range(2):
                jt = 2 * pair + o
                tp = pT.tile([128, C], F32, tag="pt", name=f"at_{b}_{jt}")
                for it in range(CT):
                    nc.tensor.matmul(
                        tp[:, it * 128:(it + 1) * 128],
                        s["att"][it][:, jt * 128:(jt + 1) * 128],
                        s["D"][it][:],
                        start=True,
                        stop=True,
                    )
                if jt % 2 == 0:
                    nc.vector.tensor_copy(s["attT8"][pair][:, o, :], tp[:])
                else:
                    nc.scalar.copy(s["attT8"][pair][:, o, :], tp[:])

        MM2_BANKS4 = ["po", "po", "pt", "pt"]
        MM2_BANKS8 = ["po", "po", "pt", "pt", "pe", "pe", "pe", "pe"]

        def emit_mm2_chunk(b, it, nch, ci, banks=None):
            # out[it, nch] = gamma*(att @ xf)[it, nch] + x[it, nch]:
            #   2 fp8 DoubleRow matmuls (c-tile pairs) accumulate the
            #   attention part in PSUM; the residual rides the eviction
            #   as a DVE tensor_add with the bf16 xf tile.
            # rotate over PSUM banks (pO's 2 + pT's 2 + retired pE's 4
            # when free) so matmuls never wait on eviction latency
            s = st[b]
            tag = banks[ci % len(banks)] if banks else "po"
            pool = {"po": pO, "pt": pT, "pe": pE}[tag]
            po = pool.tile([128, 512], F32, tag=tag,
                           name=f"po_{b}_{it}_{nch}")
            for a in range(NP):
                nc.tensor.matmul(
                    po[:],
                    s["attT8"][a][:, :, it * 128:(it + 1) * 128],
                    s["xf8"][a][:, :, nch * 512:(nch + 1) * 512],
                    start=(a == 0),
                    stop=(a == NP - 1),
                    perf_mode=DR,
                )
            o_t = out_pool.tile([128, 512], F32, tag="o",
                                name=f"o_{b}_{it}_{nch}")
            nc.vector.tensor_add(
                o_t[:], po[:], xf_slice(b, it, nch * 512, 512))
            nc.sync.dma_start(
                o_d[b, it * 128:(it + 1) * 128,
                    nch * 512:(nch + 1) * 512],
                o_t[:],
            )

        def emit_mm2(b, chunks=None, interleave=None, banks=MM2_BANKS4):
            for ci, (it, nch) in enumerate(
                    chunks if chunks is not None else
                    [(i, n) for i in range(CT) for n in range(N // 512)]):
                emit_mm2_chunk(b, it, nch, ci, banks=banks)
                if interleave is not None:
                    interleave(ci)

        # ---- emission schedule ----
        emit_loads(0)
        emit_loads(1)
        alloc_xf8(0)
        alloc_xf8(1)

        # b0's fp8 casts ride inside b0's tr+MM1 phase, each chunk cast
        # just after its loads land
        B0_CAST_AT = {2: 0, 6: 1, 12: 2, 18: 3, 24: 4}

        def ilv_b0(k):
            if k in B0_CAST_AT:
                emit_cast8(0, B0_CAST_AT[k])

        emit_trmm1(0, interleave=ilv_b0)
        emit_mirror(0)
        emit_softmax(0, 0)  # e[0] needs no mirror; unblocks b1's MM1 early
        pfx = [emit_tr(1, k) for k in range(PFX)]

        ALL_CHUNKS = [(i, n) for i in range(CT) for n in range(N // 512)]

        B1_CAST_AT = {10: 0, 13: 1, 18: 2, 24: 3, 29: 4}

        def ilv_sm0(k):
            # softmax(0), attT8(0), then the first mm2(0) chunks ride
            # inside b1's tr+MM1 phase: PE has buffered mm2 work to chew
            # while the MM1 tail waits on b1's final (DMA-starved) loads,
            # and the output store stream starts before loads fully drain.
            # b1's fp8 casts trail b1's load chunks.
            if PFX + 1 <= k <= PFX + 3:
                emit_softmax(0, k - PFX)
            elif k in (11, 12):
                emit_attT8(0, k - 11)
            elif k >= 14:
                it, nch = ALL_CHUNKS[k - 14]
                emit_mm2_chunk(0, it, nch, ci=k, banks=["po"])
            if k in B1_CAST_AT:
                emit_cast8(1, B1_CAST_AT[k])

        emit_trmm1(1, k_from=PFX, prefix=pfx, interleave=ilv_sm0)
        emit_mirror(1)

        def ilv_sm1(ci):
            # softmax(1) rides inside b0's MM2 phase
            if 1 <= ci <= 4:
                emit_softmax(1, ci - 1)

        def ilv_sm1b(ci):
            # attT8(1) once all of b1's energy has been consumed
            if ci in (0, 2):
                emit_attT8(1, ci // 2)

        emit_mm2(0, chunks=ALL_CHUNKS[18:26], interleave=ilv_sm1,
                 banks=MM2_BANKS4)
        emit_mm2(0, chunks=ALL_CHUNKS[26:], interleave=ilv_sm1b,
                 banks=MM2_BANKS8)
        emit_mm2(1, banks=MM2_BANKS8)

    nc.compile()
    return nc


_RUNNER = None


def _build_runner(nc=None):
    """Compile once; return a callable (xf_full, gamma) -> out_full.

    Mirrors concourse.bass2jax.run_bass_via_pjrt but caches the jitted
    shard_map executable so repeated kernel() calls don't re-lower, and
    keeps the output-seed zero buffers resident on device.
    """
    import jax
    from jax.sharding import Mesh, NamedSharding, PartitionSpec
    from jax.experimental.shard_map import shard_map

    from concourse import bass2jax, mybir as _mybir
    from concourse.bass2jax import _bass_exec_p, partition_id_tensor

    if nc is None:
        nc = _build_nc()
    bass2jax.install_neuronx_cc_hook()

    partition_name = (
        nc.partition_id_tensor.name if nc.partition_id_tensor else None
    )
    in_names, out_names, out_avals, zero_shapes = [], [], [], []
    for alloc in nc.m.functions[0].allocations:
        if not isinstance(alloc, _mybir.MemoryLocationSet):
            continue
        name = alloc.memorylocations[0].name
        if alloc.kind == "ExternalInput":
            if name != partition_name:
                in_names.append(name)
        elif alloc.kind == "ExternalOutput":
            shape = tuple(alloc.tensor_shape)
            dtype = _mybir.dt.np(alloc.dtype)
            out_names.append(name)
            out_avals.append(jax.core.ShapedArray(shape, dtype))
            zero_shapes.append((shape, dtype))
    n_params = len(in_names)
    all_names = list(in_names) + list(out_names)
    if partition_name is not None:
        all_names.append(partition_name)

    def _body(*args):
        operands = list(args)
        if partition_name is not None:
            operands.append(partition_id_tensor())
        return tuple(
            _bass_exec_p.bind(
                *operands,
                out_avals=tuple(out_avals),
                in_names=tuple(all_names),
                out_names=tuple(out_names),
                lowering_input_output_aliases=(),
                sim_require_finite=True,
                sim_require_nnan=True,
                nc=nc,
            )
        )

    devices = jax.devices()[:N_CORES]
    mesh = Mesh(np.asarray(devices), ("core",))
    n_in = n_params + len(out_names)
    sharded = jax.jit(
        shard_map(
            _body,
            mesh=mesh,
            in_specs=(PartitionSpec("core"),) * n_in,
            out_specs=(PartitionSpec("core"),) * len(out_names),
            check_rep=False,
        ),
        keep_unused=True,
    )

    # in_names order is discovered from allocations; map our two inputs
    assert set(in_names) == {"x", "gamma"}, in_names

    # output-seed buffers created on device once (kernel writes out fully)
    sh = NamedSharding(mesh, PartitionSpec("core"))
    zeros_dev = [
        jax.jit(
            lambda s=s, d=d: jax.numpy.zeros((N_CORES * s[0],) + s[1:], d),
            out_shardings=sh,
        )()
        for s, d in zero_shapes
    ]
    jax.block_until_ready(zeros_dev)

    def run(xf_full, gamma):
        per_in = {
            "x": xf_full,  # (16, 512, 4096) == concat of per-core (2, 512, 4096)
            "gamma": np.ascontiguousarray(
                np.broadcast_to(np.asarray(gamma, np.float32).reshape(1),
                                (N_CORES,))
            ),
        }
        concat_in = [per_in[name] for name in in_names]
        out_arrs = sharded(*concat_in, *zeros_dev)
        return np.asarray(out_arrs[out_names.index("out")])

    run.sharded = sharded
    run.zeros_dev = zeros_dev
    run.in_names = in_names
    run.out_names = out_names
    run.mesh = mesh
    return run


def _get_runner():
    global _RUNNER
    if _RUNNER is None:
        _RUNNER = _build_runner()
    return _RUNNER


def kernel(x, gamma):
    assert x.shape == (B, C, H, W)
    run = _get_runner()
    xf = np.ascontiguousarray(np.asarray(x, np.float32).reshape(B, C, N))
    g = np.asarray(gamma, np.float32)
    out = run(xf, g)
    return out.reshape(B, C, H, W).astype(np.float32, copy=False)


# revision 22
# speedup vs baseline: 1.6044x; 1.0339x over previous
"""CAM_Module (channel attention) Trainium2 Bass kernel, v3 (fp8 MM2).

x: (16, 512, 64, 64) f32, gamma: (1,) f32
  xf = x.reshape(B, C, N)           N = 4096
  energy = xf @ xf^T                (B, C, C)
  att = softmax(max(energy) - energy, axis=-1)   == softmax(-energy) (shift-invariant)
  out = gamma * (att @ xf) + x

Sharding: data-parallel over batch, 2 batches per core on 8 cores.

v3 design (vs v2):
  - MM2 runs fp8e4 DoubleRow (2 matmuls/chunk over paired c-tiles), cutting
    MM2 PE time ~1.4x; PE total drops below the ~94us HBM roofline
  - xf is cast bf16 -> fp8 pair tiles on GpSimd (idle on HW) per load chunk
  - attT drops the folded +I: residual now rides the PSUM->SBUF eviction as
    a DVE tensor_add(psum, xf_bf16) -> f32, same cost as the old copy
  - aT stored as fp8 [128, 2, C] pair tiles = DoubleRow lhsT layout
  - store stream starts mid b1-MM1 (~when loads drain) so the DMA queues
    stay saturated end to end
"""

import sys

if "/opt/trn_rl_repo" not in sys.path:
    sys.path.insert(0, "/opt/trn_rl_repo")

from contextlib import ExitStack

import numpy as np

import concourse.bass as bass
import concourse.tile as tile
from concourse import bacc, mybir
from concourse.masks import make_identity

N_CORES = 8
B, C, H, W = 16, 512, 64, 64
N = H * W                    # 4096
BPC = B // N_CORES           # batches per core = 2
CT = C // 128                # 4 c-tiles
NP = CT // 2                 # c-tile pairs (DoubleRow contraction)
KT = N // 128                # 32 k-chunks (transposed layout)

F32 = mybir.dt.float32
F32R = mybir.dt.float32r
BF16 = mybir.dt.bfloat16
F8 = mybir.dt.float8e4
DR = mybir.MatmulPerfMode.DoubleRow

LAG = 4                      # tr -> MM1 pipeline depth (k-chunks)
PFX = 6                      # b1 transposes emitted before softmax(0)


def _build_nc(reps=1):
    nc = bacc.Bacc("TRN2", target_bir_lowering=False, debug=False,
                   num_devices=N_CORES)
    x_d = nc.dram_tensor("x", [BPC, C, N], F32, kind="ExternalInput").ap()
    g_d = nc.dram_tensor("gamma", [1], F32, kind="ExternalInput").ap()
    o_d = nc.dram_tensor("out", [BPC, C, N], F32, kind="ExternalOutput").ap()

    with tile.TileContext(nc) as tc, ExitStack() as ctx:
        xf_pool = ctx.enter_context(tc.tile_pool(name="xf", bufs=BPC * CT))
        xf8_pool = ctx.enter_context(tc.tile_pool(name="xf8", bufs=BPC * NP))
        xfT_pool = ctx.enter_context(tc.tile_pool(name="xfT", bufs=LAG + 4))
        att_pool = ctx.enter_context(tc.tile_pool(name="att", bufs=2 * CT))
        attT_pool = ctx.enter_context(tc.tile_pool(name="attT", bufs=2 * NP))
        d_pool = ctx.enter_context(tc.tile_pool(name="dsc", bufs=2 * CT))
        mir_pool = ctx.enter_context(tc.tile_pool(name="mir", bufs=3))
        out_pool = ctx.enter_context(tc.tile_pool(name="outp", bufs=28))
        stat_pool = ctx.enter_context(tc.tile_pool(name="stat", bufs=4 * CT))
        one_pool = ctx.enter_context(tc.tile_pool(name="one", bufs=1))
        pT = ctx.enter_context(tc.tile_pool(name="pT", bufs=2, space="PSUM"))
        pE = ctx.enter_context(tc.tile_pool(name="pE", bufs=CT, space="PSUM"))
        pO = ctx.enter_context(tc.tile_pool(name="pO", bufs=2, space="PSUM"))

        # identities for PE transpose-mode: f32 master, bf16 + f32r copies
        ident_f = one_pool.tile([128, 128], F32, tag="idf")
        make_identity(nc, ident_f[:])
        ident = one_pool.tile([128, 128], BF16, tag="idb")
        nc.vector.tensor_copy(ident[:], ident_f[:])
        ident_r = one_pool.tile([128, 128], F32R, tag="idr")
        nc.vector.tensor_copy(ident_r[:], ident_f[:])

        # HAM warmup: ~2.5us of dummy matmuls fill the initial DMA wait
        # and bring the PE clock gate to 8/8 before the real transposes.
        # Outside the rep loop: across reps PE idle gaps stay < 3.4us so
        # the clock gate never re-throttles.
        wu = pT.tile([128, 128], F32, tag="pt", name="wu")
        for i in range(24):
            nc.tensor.matmul(wu[:], ident[:], ident[:], start=True,
                             stop=True)

        # broadcast gamma to all 128 partitions via K=1 matmul with ones
        # (after the warmup so the ~2.4us g_sb DMA doesn't head-block PE)
        g_sb = one_pool.tile([1, 1], F32, tag="gsb")
        nc.sync.dma_start(g_sb[:], g_d.rearrange("(a b) -> a b", a=1))
        ones = one_pool.tile([1, 128], F32, tag="ones")
        nc.vector.memset(ones[:], 1.0)
        pG = pT.tile([128, 1], F32, tag="pt", name="pG")
        nc.tensor.matmul(pG[:], ones[:], g_sb[:], start=True, stop=True)
        g_bc = one_pool.tile([128, 1], F32, tag="gbc")
        nc.vector.tensor_copy(g_bc[:], pG[:])

        loop_ctx = tc.For_i(0, reps, 1) if reps > 1 else None
        if loop_ctx is not None:
            ctx.enter_context(loop_ctx)

        # per-c-tile load chunks; chunk 0 is issued as two sub-DMAs
        # (128 + 384 cols) so the first transposes can start early.
        # SWDGE has ~1us fixed cost per dma_start, so later chunks are big.
        CHUNKS = [(0, 512), (512, 512), (1024, 1024), (2048, 1024),
                  (3072, 1024)]

        def chunk_of(col):
            for i, (off, w) in enumerate(CHUNKS):
                if off <= col < off + w:
                    return i, col - off
            raise AssertionError(col)

        st = [dict() for _ in range(BPC)]

        def emit_loads(b):
            s = st[b]
            s["xf"] = [[None] * len(CHUNKS) for _ in range(CT)]
            for q in range(len(CHUNKS)):
                off, w = CHUNKS[q]
                for ct in range(CT):
                    t = xf_pool.tile([128, w], BF16, tag=f"xf{q}",
                                     name=f"xf_{b}_{ct}_{q}")
                    if q == 0:
                        nc.gpsimd.dma_start(
                            t[:, 0:128],
                            x_d[b, ct * 128:(ct + 1) * 128, 0:128])
                        nc.gpsimd.dma_start(
                            t[:, 128:512],
                            x_d[b, ct * 128:(ct + 1) * 128, 128:512])
                    else:
                        nc.gpsimd.dma_start(
                            t[:],
                            x_d[b, ct * 128:(ct + 1) * 128, off:off + w])
                    s["xf"][ct][q] = t

        def alloc_xf8(b):
            # xf8[a] = fp8 pair tile [128, 2, N]: o=0 -> c-tile 2a,
            # o=1 -> c-tile 2a+1 (DoubleRow rhs pair layout)
            st[b]["xf8"] = [
                xf8_pool.tile([128, 2, N], F8, tag="xf8",
                              name=f"xf8_{b}_{a}")
                for a in range(NP)
            ]

        def emit_cast8(b, q):
            # cast one load chunk to fp8 across all 4 c-tiles, split
            # DVE/ACT; emitted right after that chunk's loads land so the
            # casts never head-block either queue
            s = st[b]
            off, w = CHUNKS[q]
            for ct in range(CT):
                a, o = divmod(ct, 2)
                dst = s["xf8"][a][:, o, off:off + w]
                if ct % 2 == 0:
                    nc.vector.tensor_copy(dst, s["xf"][ct][q][:])
                else:
                    nc.scalar.copy(dst, s["xf"][ct][q][:])

        def xf_slice(b, ct, col, width):
            q, o = chunk_of(col)
            return st[b]["xf"][ct][q][:, o:o + width]

        def emit_tr(b, k):
            tp = pT.tile([128, C], BF16, tag="pt", name=f"tp_{b}_{k}")
            for ct in range(CT):
                nc.tensor.transpose(
                    tp[:, ct * 128:(ct + 1) * 128],
                    xf_slice(b, ct, k * 128, 128),
                    ident[:],
                )
            xT = xfT_pool.tile([128, C], BF16, tag="xT", name=f"xT_{b}_{k}")
            if k % 2 == 0:
                nc.vector.tensor_copy(xT[:], tp[:])
            else:
                nc.scalar.copy(xT[:], tp[:])
            return xT

        def emit_mm1(b, k, xT):
            # energy is symmetric: compute only j >= i blocks (shrinking
            # moving width per i-tile); lower blocks are mirrored after
            for it in range(CT):
                nc.tensor.matmul(
                    st[b]["e"][it][:, it * 128:C],
                    xT[:, it * 128:(it + 1) * 128],
                    xT[:, it * 128:C],
                    start=(k == 0),
                    stop=(k == KT - 1),
                )

        def emit_trmm1(b, k_from=0, prefix=(), interleave=None):
            s = st[b]
            s["e"] = [
                pE.tile([128, C], F32, tag="pe", name=f"pe_{b}_{i}")
                for i in range(CT)
            ]
            pending = list(prefix)
            for k in range(k_from, KT):
                pending.append(emit_tr(b, k))
                if len(pending) > LAG:
                    emit_mm1(b, k - len(pending) + 1, pending.pop(0))
                if interleave is not None:
                    interleave(k)
            base = KT - len(pending)
            for i, xT in enumerate(pending):
                emit_mm1(b, base + i, xT)

        def emit_mirror(b):
            # mirror lower-triangle blocks e[t][:, u] = e[u][:, t].T via
            # sbuf bounce + transpose into a scratch psum bank + ACT
            # write-back (PE never touches accumulation-grouped banks);
            # f32r keeps the mirrored energies bit-exact
            e_ps = st[b]["e"]
            for t in range(1, CT):
                mp = pT.tile([128, C], F32R, tag="pt", name=f"mp_{b}_{t}")
                for u in range(t):
                    mtmp = mir_pool.tile([128, 128], F32R, tag="mir",
                                         name=f"mir_{b}_{t}_{u}")
                    nc.scalar.copy(
                        mtmp[:], e_ps[u][:, t * 128:(t + 1) * 128])
                    nc.tensor.transpose(
                        mp[:, u * 128:(u + 1) * 128], mtmp[:], ident_r[:])
                nc.scalar.copy(
                    e_ps[t][:, 0:t * 128], mp[:, 0:t * 128])

        def emit_softmax(b, it):
            # att row i = exp(min_i - e_i); the 1/Z*gamma scale is deferred
            # to the attT transpose via D = diag(rz*g)
            s = st[b]
            if it == 0:
                s["att"] = [None] * CT
                s["D"] = [None] * CT
            m = stat_pool.tile([128, 1], F32, tag="m", name=f"m_{b}_{it}")
            nc.vector.tensor_reduce(
                m[:], s["e"][it][:], axis=mybir.AxisListType.X,
                op=mybir.AluOpType.min,
            )
            a = att_pool.tile([128, C], BF16, tag="a", name=f"a_{b}_{it}")
            z = stat_pool.tile([128, 1], F32, tag="z", name=f"z_{b}_{it}")
            nc.scalar.activation(
                a[:], s["e"][it][:], mybir.ActivationFunctionType.Exp,
                bias=m[:], scale=-1.0, accum_out=z[:],
            )
            rz = stat_pool.tile([128, 1], F32, tag="rz", name=f"rz_{b}_{it}")
            nc.vector.reciprocal(rz[:], z[:])
            g = stat_pool.tile([128, 1], F32, tag="g", name=f"g_{b}_{it}")
            nc.vector.tensor_mul(g[:], rz[:], g_bc[:])
            D = d_pool.tile([128, 128], BF16, tag="D", name=f"D_{b}_{it}")
            nc.vector.tensor_scalar_mul(D[:], ident[:], g[:])
            s["att"][it] = a
            s["D"][it] = D

        def emit_attT8(b, pair):
            # aT8[a][:, o, :] = (att.T * colscale) for c-tile jt = 2a+o,
            # cast to fp8 = DoubleRow lhsT pair layout.  No +I fold: the
            # residual is added at PSUM eviction in emit_mm2_chunk.
            s = st[b]
            if "attT8" not in s:
                s["attT8"] = [
                    attT_pool.tile([128, 2, C], F8, tag="aT8",
                                   name=f"aT8_{b}_{aa}")
                    for aa in range(NP)
                ]
            for o in range(2):
                jt = 2 * pair + o
                tp = pT.tile([128, C], F32, tag="pt", name=f"at_{b}_{jt}")
                for it in range(CT):
                    nc.tensor.matmul(
                        tp[:, it * 128:(it + 1) * 128],
                        s["att"][it][:, jt * 128:(jt + 1) * 128],
                        s["D"][it][:],
                        start=True,
                        stop=True,
                    )
                if jt % 2 == 0:
                    nc.vector.tensor_copy(s["attT8"][pair][:, o, :], tp[:])
                else:
                    nc.scalar.copy(s["attT8"][pair][:, o, :], tp[:])

        MM2_BANKS4 = ["po", "po", "pt", "pt"]
        MM2_BANKS8 = ["po", "po", "pt", "pt", "pe", "pe", "pe", "pe"]

        def emit_mm2_chunk(b, it, nch, ci, banks=None):
            # out[it, nch] = gamma*(att @ xf)[it, nch] + x[it, nch]:
            #   2 fp8 DoubleRow matmuls (c-tile pairs) accumulate the
            #   attention part in PSUM; the residual rides the eviction
            #   as a DVE tensor_add with the bf16 xf tile.
            # rotate over PSUM banks (pO's 2 + pT's 2 + retired pE's 4
            # when free) so matmuls never wait on eviction latency
            s = st[b]
            tag = banks[ci % len(banks)] if banks else "po"
            pool = {"po": pO, "pt": pT, "pe": pE}[tag]
            po = pool.tile([128, 512], F32, tag=tag,
                           name=f"po_{b}_{it}_{nch}")
            for a in range(NP):
                nc.tensor.matmul(
                    po[:],
                    s["attT8"][a][:, :, it * 128:(it + 1) * 128],
                    s["xf8"][a][:, :, nch * 512:(nch + 1) * 512],
                    start=(a == 0),
                    stop=(a == NP - 1),
                    perf_mode=DR,
                )
            o_t = out_pool.tile([128, 512], F32, tag="o",
                                name=f"o_{b}_{it}_{nch}")
            nc.vector.tensor_add(
                o_t[:], po[:], xf_slice(b, it, nch * 512, 512))
            nc.sync.dma_start(
                o_d[b, it * 128:(it + 1) * 128,
                    nch * 512:(nch + 1) * 512],
                o_t[:],
            )

        def emit_mm2(b, chunks=None, interleave=None, banks=MM2_BANKS4):
            for ci, (it, nch) in enumerate(
                    chunks if chunks is not None else
                    [(i, n) for i in range(CT) for n in range(N // 512)]):
                emit_mm2_chunk(b, it, nch, ci, banks=banks)
                if interleave is not None:
                    interleave(ci)

        # ---- emission schedule ----
        emit_loads(0)
        emit_loads(1)
        alloc_xf8(0)
        alloc_xf8(1)

        # store gate: a trailing 1-element f32 load rides the SWDGE queue
        # right behind the last load chunk; the 1-element store below
        # depends on it and heads the sync queue, so the whole store
        # stream (FIFO behind it) issues only once loads have drained.
        # Loads and stores then never contend for the shared ~315 GB/s
        # HBM path, and b1's MM1 tail is never starved by store traffic.
        # The real store of that region overwrites the gate value later
        # (same queue, FIFO order).
        gate_t = one_pool.tile([1, 1], F32, tag="gate", bufs=2)
        nc.gpsimd.dma_start(gate_t[:], x_d[1, C - 1:C, N - 1:N])
        nc.sync.dma_start(o_d[1, C - 1:C, N - 1:N], gate_t[:])



        # b0's fp8 casts ride inside b0's tr+MM1 phase, each chunk cast
        # just after its loads land
        B0_CAST_AT = {2: 0, 6: 1, 12: 2, 18: 3, 24: 4}

        def ilv_b0(k):
            if k in B0_CAST_AT:
                emit_cast8(0, B0_CAST_AT[k])

        emit_trmm1(0, interleave=ilv_b0)
        emit_mirror(0)
        emit_softmax(0, 0)  # e[0] needs no mirror; unblocks b1's MM1 early
        pfx = [emit_tr(1, k) for k in range(PFX)]

        ALL_CHUNKS = [(i, n) for i in range(CT) for n in range(N // 512)]

        B1_CAST_AT = {10: 0, 13: 1, 18: 2, 24: 3, 29: 4}

        def ilv_sm0(k):
            # softmax(0), attT8(0), then the first mm2(0) chunks ride
            # inside b1's tr+MM1 phase: PE has buffered mm2 work to chew
            # while the MM1 tail waits on b1's final (DMA-starved) loads,
            # and the output store stream starts before loads fully drain.
            # b1's fp8 casts trail b1's load chunks.
            if PFX + 1 <= k <= PFX + 3:
                emit_softmax(0, k - PFX)
            elif k in (14, 15):
                emit_attT8(0, k - 14)
            elif k >= 16:
                it, nch = ALL_CHUNKS[k - 16]
                emit_mm2_chunk(0, it, nch, ci=k, banks=["po"])
            if k in B1_CAST_AT:
                emit_cast8(1, B1_CAST_AT[k])

        emit_trmm1(1, k_from=PFX, prefix=pfx, interleave=ilv_sm0)
        emit_mirror(1)

        def ilv_sm1(ci):
            # softmax(1) rides inside b0's MM2 phase
            if 1 <= ci <= 4:
                emit_softmax(1, ci - 1)

        def ilv_sm1b(ci):
            # attT8(1) once all of b1's energy has been consumed
            if ci in (0, 2):
                emit_attT8(1, ci // 2)

        emit_mm2(0, chunks=ALL_CHUNKS[16:24], interleave=ilv_sm1,
                 banks=MM2_BANKS4)
        emit_mm2(0, chunks=ALL_CHUNKS[24:], interleave=ilv_sm1b,
                 banks=MM2_BANKS8)
        emit_mm2(1, banks=MM2_BANKS8)

    nc.compile()
    return nc


_RUNNER = None


def _build_runner(nc=None):
    """Compile once; return a callable (xf_full, gamma) -> out_full.

    Mirrors concourse.bass2jax.run_bass_via_pjrt but caches the jitted
    shard_map executable so repeated kernel() calls don't re-lower, and
    keeps the output-seed zero buffers resident on device.
    """
    import jax
    from jax.sharding import Mesh, NamedSharding, PartitionSpec
    from jax.experimental.shard_map import shard_map

    from concourse import bass2jax, mybir as _mybir
    from concourse.bass2jax import _bass_exec_p, partition_id_tensor

    if nc is None:
        nc = _build_nc()
    bass2jax.install_neuronx_cc_hook()

    partition_name = (
        nc.partition_id_tensor.name if nc.partition_id_tensor else None
    )
    in_names, out_names, out_avals, zero_shapes = [], [], [], []
    for alloc in nc.m.functions[0].allocations:
        if not isinstance(alloc, _mybir.MemoryLocationSet):
            continue
        name = alloc.memorylocations[0].name
        if alloc.kind == "ExternalInput":
            if name != partition_name:
                in_names.append(name)
        elif alloc.kind == "ExternalOutput":
            shape = tuple(alloc.tensor_shape)
            dtype = _mybir.dt.np(alloc.dtype)
            out_names.append(name)
            out_avals.append(jax.core.ShapedArray(shape, dtype))
            zero_shapes.append((shape, dtype))
    n_params = len(in_names)
    all_names = list(in_names) + list(out_names)
    if partition_name is not None:
        all_names.append(partition_name)

    def _body(*args):
        operands = list(args)
        if partition_name is not None:
            operands.append(partition_id_tensor())
        return tuple(
            _bass_exec_p.bind(
                *operands,
                out_avals=tuple(out_avals),
                in_names=tuple(all_names),
                out_names=tuple(out_names),
                lowering_input_output_aliases=(),
                sim_require_finite=True,
                sim_require_nnan=True,
                nc=nc,
            )
        )

    devices = jax.devices()[:N_CORES]
    mesh = Mesh(np.asarray(devices), ("core",))
    n_in = n_params + len(out_names)
    sharded = jax.jit(
        shard_map(
            _body,
            mesh=mesh,
            in_specs=(PartitionSpec("core"),) * n_in,
            out_specs=(PartitionSpec("core"),) * len(out_names),
            check_rep=False,
        ),
        keep_unused=True,
    )

    # in_names order is discovered from allocations; map our two inputs
    assert set(in_names) == {"x", "gamma"}, in_names

    # output-seed buffers created on device once (kernel writes out fully)
    sh = NamedSharding(mesh, PartitionSpec("core"))
    zeros_dev = [
        jax.jit(
            lambda s=s, d=d: jax.numpy.zeros((N_CORES * s[0],) + s[1:], d),
            out_shardings=sh,
        )()
        for s, d in zero_shapes
    ]
    jax.block_until_ready(zeros_dev)

    def run(xf_full, gamma):
        per_in = {
            "x": xf_full,  # (16, 512, 4096) == concat of per-core (2, 512, 4096)
            "gamma": np.ascontiguousarray(
                np.broadcast_to(np.asarray(gamma, np.float32).reshape(1),
                                (N_CORES,))
            ),
        }
        concat_in = [per_in[name] for name in in_names]
        out_arrs = sharded(*concat_in, *zeros_dev)
        return np.asarray(out_arrs[out_names.index("out")])

    run.sharded = sharded
    run.zeros_dev = zeros_dev
    run.in_names = in_names
    run.out_names = out_names
    run.mesh = mesh
    return run


def _get_runner():
    global _RUNNER
    if _RUNNER is None:
        _RUNNER = _build_runner()
    return _RUNNER


def kernel(x, gamma):
    assert x.shape == (B, C, H, W)
    run = _get_runner()
    xf = np.ascontiguousarray(np.asarray(x, np.float32).reshape(B, C, N))
    g = np.asarray(gamma, np.float32)
    out = run(xf, g)
    return out.reshape(B, C, H, W).astype(np.float32, copy=False)


# revision 26
# speedup vs baseline: 1.6325x; 1.0176x over previous
"""CAM_Module (channel attention) Trainium2 Bass kernel, v3 (fp8 MM2).

x: (16, 512, 64, 64) f32, gamma: (1,) f32
  xf = x.reshape(B, C, N)           N = 4096
  energy = xf @ xf^T                (B, C, C)
  att = softmax(max(energy) - energy, axis=-1)   == softmax(-energy) (shift-invariant)
  out = gamma * (att @ xf) + x

Sharding: data-parallel over batch, 2 batches per core on 8 cores.

v3 design (vs v2):
  - MM2 runs fp8e4 DoubleRow (2 matmuls/chunk over paired c-tiles), cutting
    MM2 PE time ~1.4x; PE total drops below the ~94us HBM roofline
  - xf is cast bf16 -> fp8 pair tiles on GpSimd (idle on HW) per load chunk
  - attT drops the folded +I: residual now rides the PSUM->SBUF eviction as
    a DVE tensor_add(psum, xf_bf16) -> f32, same cost as the old copy
  - aT stored as fp8 [128, 2, C] pair tiles = DoubleRow lhsT layout
  - store stream starts mid b1-MM1 (~when loads drain) so the DMA queues
    stay saturated end to end
"""

import sys

if "/opt/trn_rl_repo" not in sys.path:
    sys.path.insert(0, "/opt/trn_rl_repo")

from contextlib import ExitStack

import numpy as np

import concourse.bass as bass
import concourse.tile as tile
from concourse import bacc, mybir
from concourse.masks import make_identity

N_CORES = 8
B, C, H, W = 16, 512, 64, 64
N = H * W                    # 4096
BPC = B // N_CORES           # batches per core = 2
CT = C // 128                # 4 c-tiles
NP = CT // 2                 # c-tile pairs (DoubleRow contraction)
KT = N // 128                # 32 k-chunks (transposed layout)

F32 = mybir.dt.float32
F32R = mybir.dt.float32r
BF16 = mybir.dt.bfloat16
F8 = mybir.dt.float8e4
DR = mybir.MatmulPerfMode.DoubleRow

XP = KT // 2                 # 16 k-chunk pairs (DoubleRow MM1)
LAG_P = 2                    # tr -> MM1 pipeline depth (pairs)
PFX_P = 3                    # b1 pairs emitted before softmax(0)

MM2_BANKS4 = ["po", "po", "pt", "pt"]
MM2_BANKS8 = ["po", "po", "pt", "pt", "pe", "pe", "pe", "pe"]


def _build_nc(reps=1):
    nc = bacc.Bacc("TRN2", target_bir_lowering=False, debug=False,
                   num_devices=N_CORES)
    x_d = nc.dram_tensor("x", [BPC, C, N], F32, kind="ExternalInput").ap()
    g_d = nc.dram_tensor("gamma", [1], F32, kind="ExternalInput").ap()
    o_d = nc.dram_tensor("out", [BPC, C, N], F32, kind="ExternalOutput").ap()

    with tile.TileContext(nc) as tc, ExitStack() as ctx:
        xf_pool = ctx.enter_context(tc.tile_pool(name="xf", bufs=BPC * CT))
        xf8_pool = ctx.enter_context(tc.tile_pool(name="xf8", bufs=BPC * NP))
        xfT_pool = ctx.enter_context(tc.tile_pool(name="xfT", bufs=4))
        att_pool = ctx.enter_context(tc.tile_pool(name="att", bufs=2 * CT))
        attT_pool = ctx.enter_context(tc.tile_pool(name="attT", bufs=2 * NP))
        d_pool = ctx.enter_context(tc.tile_pool(name="dsc", bufs=2 * CT))
        mir_pool = ctx.enter_context(tc.tile_pool(name="mir", bufs=3))
        out_pool = ctx.enter_context(tc.tile_pool(name="outp", bufs=28))
        stat_pool = ctx.enter_context(tc.tile_pool(name="stat", bufs=4 * CT))
        one_pool = ctx.enter_context(tc.tile_pool(name="one", bufs=1))
        pT = ctx.enter_context(tc.tile_pool(name="pT", bufs=2, space="PSUM"))
        pE = ctx.enter_context(tc.tile_pool(name="pE", bufs=CT, space="PSUM"))
        pO = ctx.enter_context(tc.tile_pool(name="pO", bufs=2, space="PSUM"))

        # identities for PE transpose-mode: f32 master, bf16 + f32r copies
        ident_f = one_pool.tile([128, 128], F32, tag="idf")
        make_identity(nc, ident_f[:])
        ident = one_pool.tile([128, 128], BF16, tag="idb")
        nc.vector.tensor_copy(ident[:], ident_f[:])
        ident_r = one_pool.tile([128, 128], F32R, tag="idr")
        nc.vector.tensor_copy(ident_r[:], ident_f[:])

        # HAM warmup (outside the rep loop): bring the PE clock gate to
        # 8/8 before the first real transposes; across reps PE idle gaps
        # stay < 3.4us so it never re-throttles.
        wu = pT.tile([128, 128], F32, tag="pt", name="wu")
        for i in range(24):
            nc.tensor.matmul(wu[:], ident[:], ident[:], start=True,
                             stop=True)

        # broadcast gamma to all 128 partitions via K=1 matmul with ones
        g_sb = one_pool.tile([1, 1], F32, tag="gsb")
        nc.sync.dma_start(g_sb[:], g_d.rearrange("(a b) -> a b", a=1))
        ones = one_pool.tile([1, 128], F32, tag="ones")
        nc.vector.memset(ones[:], 1.0)
        pG = pT.tile([128, 1], F32, tag="pt", name="pG")
        nc.tensor.matmul(pG[:], ones[:], g_sb[:], start=True, stop=True)
        g_bc = one_pool.tile([128, 1], F32, tag="gbc")
        nc.vector.tensor_copy(g_bc[:], pG[:])

        loop_ctx = tc.For_i(0, reps, 1) if reps > 1 else None
        if loop_ctx is not None:
            ctx.enter_context(loop_ctx)

        # load chunks; chunk 0 split (128 + 384 cols) so the first
        # transposes start early; the tail is split 512/512 so the MM1
        # tail is not gated on one big trailing DMA
        CHUNKS = [(0, 512), (512, 512), (1024, 1024), (2048, 1024),
                  (3072, 512), (3584, 512)]

        def chunk_of(col):
            for i, (off, w) in enumerate(CHUNKS):
                if off <= col < off + w:
                    return i, col - off
            raise AssertionError(col)

        st = [dict() for _ in range(BPC)]

        def emit_loads(b):
            s = st[b]
            s["xf"] = [[None] * len(CHUNKS) for _ in range(CT)]
            for q in range(len(CHUNKS)):
                off, w = CHUNKS[q]
                for ct in range(CT):
                    t = xf_pool.tile([128, w], BF16, tag=f"xf{q}",
                                     name=f"xf_{b}_{ct}_{q}")
                    if q == 0:
                        nc.gpsimd.dma_start(
                            t[:, 0:128],
                            x_d[b, ct * 128:(ct + 1) * 128, 0:128])
                        nc.gpsimd.dma_start(
                            t[:, 128:512],
                            x_d[b, ct * 128:(ct + 1) * 128, 128:512])
                    else:
                        nc.gpsimd.dma_start(
                            t[:],
                            x_d[b, ct * 128:(ct + 1) * 128, off:off + w])
                    s["xf"][ct][q] = t

        def alloc_xf8(b):
            # xf8[a] = fp8 pair tile [128, 2, N]: o=0 -> c-tile 2a,
            # o=1 -> c-tile 2a+1 (DoubleRow rhs pair layout)
            st[b]["xf8"] = [
                xf8_pool.tile([128, 2, N], F8, tag="xf8",
                              name=f"xf8_{b}_{a}")
                for a in range(NP)
            ]

        def emit_cast8(b, q):
            # cast one load chunk to fp8 across all 4 c-tiles, split
            # DVE/ACT; emitted right after that chunk's loads land so the
            # casts never head-block either queue
            s = st[b]
            off, w = CHUNKS[q]
            for ct in range(CT):
                a, o = divmod(ct, 2)
                dst = s["xf8"][a][:, o, off:off + w]
                if ct % 2 == 0:
                    nc.vector.tensor_copy(dst, s["xf"][ct][q][:])
                else:
                    nc.scalar.copy(dst, s["xf"][ct][q][:])

        def xf_slice(b, ct, col, width):
            q, o = chunk_of(col)
            return st[b]["xf"][ct][q][:, o:o + width]

        def emit_tr_pair(b, m):
            # transpose k-chunks 2m, 2m+1 and evict as one fp8 pair tile
            # [128, 2, C] -- the DoubleRow operand layout for MM1
            xT8 = xfT_pool.tile([128, 2, C], F8, tag="xT",
                                name=f"xT8_{b}_{m}")
            for o in range(2):
                k = 2 * m + o
                tp = pT.tile([128, C], BF16, tag="pt", name=f"tp_{b}_{k}")
                for ct in range(CT):
                    nc.tensor.transpose(
                        tp[:, ct * 128:(ct + 1) * 128],
                        xf_slice(b, ct, k * 128, 128),
                        ident[:],
                    )
                if o == 0:
                    nc.vector.tensor_copy(xT8[:, o, :], tp[:])
                else:
                    nc.scalar.copy(xT8[:, o, :], tp[:])
            return xT8

        def emit_mm1(b, m, xT8):
            # energy is symmetric: compute only j >= i blocks (shrinking
            # moving width per i-tile); fp8 DoubleRow contracts 256 rows
            # (two k-chunks) per pass
            for it in range(CT):
                nc.tensor.matmul(
                    st[b]["e"][it][:, it * 128:C],
                    xT8[:, :, it * 128:(it + 1) * 128],
                    xT8[:, :, it * 128:C],
                    start=(m == 0),
                    stop=(m == XP - 1),
                    perf_mode=DR,
                )

        def emit_trmm1(b, m_from=0, prefix=(), interleave=None):
            s = st[b]
            s["e"] = [
                pE.tile([128, C], F32, tag="pe", name=f"pe_{b}_{i}")
                for i in range(CT)
            ]
            pending = list(prefix)
            for m in range(m_from, XP):
                pending.append(emit_tr_pair(b, m))
                if len(pending) > LAG_P:
                    emit_mm1(b, m - len(pending) + 1, pending.pop(0))
                if interleave is not None:
                    interleave(m)
            base = XP - len(pending)
            for i, xT8 in enumerate(pending):
                emit_mm1(b, base + i, xT8)

        def emit_mirror(b):
            # mirror lower-triangle blocks e[t][:, u] = e[u][:, t].T via
            # sbuf bounce + transpose into a scratch psum bank + ACT
            # write-back; f32r keeps the mirrored energies bit-exact
            e_ps = st[b]["e"]
            for t in range(1, CT):
                mp = pT.tile([128, C], F32R, tag="pt", name=f"mp_{b}_{t}")
                for u in range(t):
                    mtmp = mir_pool.tile([128, 128], F32R, tag="mir",
                                         name=f"mir_{b}_{t}_{u}")
                    nc.scalar.copy(
                        mtmp[:], e_ps[u][:, t * 128:(t + 1) * 128])
                    nc.tensor.transpose(
                        mp[:, u * 128:(u + 1) * 128], mtmp[:], ident_r[:])
                nc.scalar.copy(
                    e_ps[t][:, 0:t * 128], mp[:, 0:t * 128])

        def emit_softmax(b, it):
            # att row i = exp(min_i - e_i); the 1/Z*gamma scale is deferred
            # to the attT transpose via D = diag(rz*g)
            s = st[b]
            if it == 0:
                s["att"] = [None] * CT
                s["D"] = [None] * CT
            m = stat_pool.tile([128, 1], F32, tag="m", name=f"m_{b}_{it}")
            nc.vector.tensor_reduce(
                m[:], s["e"][it][:], axis=mybir.AxisListType.X,
                op=mybir.AluOpType.min,
            )
            a = att_pool.tile([128, C], BF16, tag="a", name=f"a_{b}_{it}")
            z = stat_pool.tile([128, 1], F32, tag="z", name=f"z_{b}_{it}")
            nc.scalar.activation(
                a[:], s["e"][it][:], mybir.ActivationFunctionType.Exp,
                bias=m[:], scale=-1.0, accum_out=z[:],
            )
            rz = stat_pool.tile([128, 1], F32, tag="rz", name=f"rz_{b}_{it}")
            nc.vector.reciprocal(rz[:], z[:])
            g = stat_pool.tile([128, 1], F32, tag="g", name=f"g_{b}_{it}")
            nc.vector.tensor_mul(g[:], rz[:], g_bc[:])
            D = d_pool.tile([128, 128], BF16, tag="D", name=f"D_{b}_{it}")
            nc.vector.tensor_scalar_mul(D[:], ident[:], g[:])
            s["att"][it] = a
            s["D"][it] = D

        def emit_attT8(b, pair):
            # aT8[a][:, o, :] = (att.T * colscale) for c-tile jt = 2a+o,
            # cast to fp8 = DoubleRow lhsT pair layout.  No +I fold: the
            # residual is added at PSUM eviction in the MM2 groups.
            s = st[b]
            if "attT8" not in s:
                s["attT8"] = [
                    attT_pool.tile([128, 2, C], F8, tag="aT8",
                                   name=f"aT8_{b}_{aa}")
                    for aa in range(NP)
                ]
            for o in range(2):
                jt = 2 * pair + o
                tp = pT.tile([128, C], F32, tag="pt", name=f"at_{b}_{jt}")
                for it in range(CT):
                    nc.tensor.matmul(
                        tp[:, it * 128:(it + 1) * 128],
                        s["att"][it][:, jt * 128:(jt + 1) * 128],
                        s["D"][it][:],
                        start=True,
                        stop=True,
                    )
                if jt % 2 == 0:
                    nc.vector.tensor_copy(s["attT8"][pair][:, o, :], tp[:])
                else:
                    nc.scalar.copy(s["attT8"][pair][:, o, :], tp[:])

        def emit_mm2_group(b, it, nchs, banks, bank_off=0):
            # out[it, nch] = gamma*(att @ xf)[it, nch] + x[it, nch] for a
            # group of nch chunks sharing the same stationary aT8 slices:
            # two passes (a = 0 start, a = 1 stop) amortize the DoubleRow
            # LDWEIGHTS over the whole group.  The residual rides the PSUM
            # eviction as a DVE tensor_add with the bf16 xf tile.
            s = st[b]
            pos = []
            for j, nch in enumerate(nchs):
                tag = banks[(bank_off + j) % len(banks)]
                pool = {"po": pO, "pt": pT, "pe": pE}[tag]
                pos.append(pool.tile([128, 512], F32, tag=tag,
                                     name=f"po_{b}_{it}_{nch}"))
            for j, nch in enumerate(nchs):
                nc.tensor.matmul(
                    pos[j][:],
                    s["attT8"][0][:, :, it * 128:(it + 1) * 128],
                    s["xf8"][0][:, :, nch * 512:(nch + 1) * 512],
                    start=True, stop=False, perf_mode=DR,
                )
            for j, nch in enumerate(nchs):
                nc.tensor.matmul(
                    pos[j][:],
                    s["attT8"][1][:, :, it * 128:(it + 1) * 128],
                    s["xf8"][1][:, :, nch * 512:(nch + 1) * 512],
                    start=False, stop=True, perf_mode=DR,
                )
                o_t = out_pool.tile([128, 512], F32, tag="o",
                                    name=f"o_{b}_{it}_{nch}")
                nc.vector.tensor_add(
                    o_t[:], pos[j][:], xf_slice(b, it, nch * 512, 512))
                nc.sync.dma_start(
                    o_d[b, it * 128:(it + 1) * 128,
                        nch * 512:(nch + 1) * 512],
                    o_t[:],
                )

        # ---- emission schedule ----
        emit_loads(0)
        emit_loads(1)
        alloc_xf8(0)
        alloc_xf8(1)

        # store gate: a trailing 1-element f32 load rides the SWDGE queue
        # right behind the last load chunk; the 1-element store below
        # depends on it and heads the sync queue, so the whole store
        # stream (FIFO behind it) issues only once loads have drained --
        # loads and stores never contend for the shared ~315 GB/s HBM
        # path, and b1 MM1 is never starved by store traffic.  The real
        # store of that region overwrites the gate value later (same
        # queue, FIFO order).
        gate_t = one_pool.tile([1, 1], F32, tag="gate", bufs=2)
        nc.gpsimd.dma_start(gate_t[:], x_d[1, C - 1:C, N - 1:N])
        nc.sync.dma_start(o_d[1, C - 1:C, N - 1:N], gate_t[:])

        # b0 casts ride inside b0 front, each chunk just after its loads
        B0_CAST_AT = {1: 0, 3: 1, 6: 2, 10: 3, 13: 4, 15: 5}

        def ilv_b0(m):
            if m in B0_CAST_AT:
                emit_cast8(0, B0_CAST_AT[m])

        emit_trmm1(0, interleave=ilv_b0)
        emit_mirror(0)
        emit_softmax(0, 0)  # e[0] needs no mirror; unblocks b1 MM1 early
        pfx = [emit_tr_pair(1, m) for m in range(PFX_P)]

        ALL_NCH = list(range(N // 512))
        B1_CAST_AT = {5: 0, 7: 1, 9: 2, 12: 3, 14: 4, 15: 5}

        def ilv_sm0(m):
            # softmax(0) + attT8(0) ride inside b1 front (ACT/DVE work,
            # tiny PE); b1 casts trail b1 load chunks
            if PFX_P + 1 <= m <= PFX_P + 3:
                emit_softmax(0, m - PFX_P)
            elif m in (8, 10):
                emit_attT8(0, (m - 8) // 2)
            elif m in (11, 13):
                # pull a couple of b0 MM2 groups into b1 front idle
                n0 = m - 11  # m=11 -> chunks 0:2, m=13 -> chunks 2:4
                emit_mm2_group(0, 0, ALL_NCH[n0:n0 + 2], banks=["po"])
            if m in B1_CAST_AT:
                emit_cast8(1, B1_CAST_AT[m])

        emit_trmm1(1, m_from=PFX_P, prefix=pfx, interleave=ilv_sm0)

        # b1 energy post-processing first so the ACT exp chain overlaps
        # b0 MM2 on the PE
        emit_mirror(1)
        emit_softmax(1, 0)

        # rest of b0 MM2: it-major groups of 4 over po+pt banks, with
        # b1 softmax / attT8 interleaved
        emit_mm2_group(0, 0, ALL_NCH[4:8], banks=MM2_BANKS4)
        emit_softmax(1, 1)
        emit_mm2_group(0, 1, ALL_NCH[0:4], banks=MM2_BANKS4)
        emit_softmax(1, 2)
        emit_mm2_group(0, 1, ALL_NCH[4:8], banks=MM2_BANKS4)
        emit_softmax(1, 3)
        emit_mm2_group(0, 2, ALL_NCH[0:4], banks=MM2_BANKS4)
        emit_attT8(1, 0)
        emit_mm2_group(0, 2, ALL_NCH[4:8], banks=MM2_BANKS4)
        emit_attT8(1, 1)
        emit_mm2_group(0, 3, ALL_NCH[0:4], banks=MM2_BANKS4)
        emit_mm2_group(0, 3, ALL_NCH[4:8], banks=MM2_BANKS4)

        # b1 MM2: it-major groups of 8 over all retired banks
        for it in range(CT):
            emit_mm2_group(1, it, ALL_NCH, banks=MM2_BANKS8)

    nc.compile()
    return nc


_RUNNER = None


def _build_runner(nc=None):
    """Compile once; return a callable (xf_full, gamma) -> out_full.

    Mirrors concourse.bass2jax.run_bass_via_pjrt but caches the jitted
    shard_map executable so repeated kernel() calls don't re-lower, and
    keeps the output-seed zero buffers resident on device.
    """
    import jax
    from jax.sharding import Mesh, NamedSharding, PartitionSpec
    from jax.experimental.shard_map import shard_map

    from concourse import bass2jax, mybir as _mybir
    from concourse.bass2jax import _bass_exec_p, partition_id_tensor

    if nc is None:
        nc = _build_nc()
    bass2jax.install_neuronx_cc_hook()

    partition_name = (
        nc.partition_id_tensor.name if nc.partition_id_tensor else None
    )
    in_names, out_names, out_avals, zero_shapes = [], [], [], []
    for alloc in nc.m.functions[0].allocations:
        if not isinstance(alloc, _mybir.MemoryLocationSet):
            continue
        name = alloc.memorylocations[0].name
        if alloc.kind == "ExternalInput":
            if name != partition_name:
                in_names.append(name)
        elif alloc.kind == "ExternalOutput":
            shape = tuple(alloc.tensor_shape)
            dtype = _mybir.dt.np(alloc.dtype)
            out_names.append(name)
            out_avals.append(jax.core.ShapedArray(shape, dtype))
            zero_shapes.append((shape, dtype))
    n_params = len(in_names)
    all_names = list(in_names) + list(out_names)
    if partition_name is not None:
        all_names.append(partition_name)

    def _body(*args):
        operands = list(args)
        if partition_name is not None:
            operands.append(partition_id_tensor())
        return tuple(
            _bass_exec_p.bind(
                *operands,
                out_avals=tuple(out_avals),
                in_names=tuple(all_names),
                out_names=tuple(out_names),
                lowering_input_output_aliases=(),
                sim_require_finite=True,
                sim_require_nnan=True,
                nc=nc,
            )
        )

    devices = jax.devices()[:N_CORES]
    mesh = Mesh(np.asarray(devices), ("core",))
    n_in = n_params + len(out_names)
    sharded = jax.jit(
        shard_map(
            _body,
            mesh=mesh,
            in_specs=(PartitionSpec("core"),) * n_in,
            out_specs=(PartitionSpec("core"),) * len(out_names),
            check_rep=False,
        ),
        keep_unused=True,
    )

    # in_names order is discovered from allocations; map our two inputs
    assert set(in_names) == {"x", "gamma"}, in_names

    # output-seed buffers created on device once (kernel writes out fully)
    sh = NamedSharding(mesh, PartitionSpec("core"))
    zeros_dev = [
        jax.jit(
            lambda s=s, d=d: jax.numpy.zeros((N_CORES * s[0],) + s[1:], d),
            out_shardings=sh,
        )()
        for s, d in zero_shapes
    ]
    jax.block_until_ready(zeros_dev)

    def run(xf_full, gamma):
        per_in = {
            "x": xf_full,  # (16, 512, 4096) == concat of per-core (2, 512, 4096)
            "gamma": np.ascontiguousarray(
                np.broadcast_to(np.asarray(gamma, np.float32).reshape(1),
                                (N_CORES,))
            ),
        }
        concat_in = [per_in[name] for name in in_names]
        out_arrs = sharded(*concat_in, *zeros_dev)
        return np.asarray(out_arrs[out_names.index("out")])

    run.sharded = sharded
    run.zeros_dev = zeros_dev
    run.in_names = in_names
    run.out_names = out_names
    run.mesh = mesh
    return run


def _get_runner():
    global _RUNNER
    if _RUNNER is None:
        _RUNNER = _build_runner()
    return _RUNNER


def kernel(x, gamma):
    assert x.shape == (B, C, H, W)
    run = _get_runner()
    xf = np.ascontiguousarray(np.asarray(x, np.float32).reshape(B, C, N))
    g = np.asarray(gamma, np.float32)
    out = run(xf, g)
    return out.reshape(B, C, H, W).astype(np.float32, copy=False)
